# revision 1
# baseline (speedup 1.0000x reference)
"""AttnBlock (GroupNorm + single-head self-attention + proj + residual) on 8 trn2 cores.

Sharding: core = (batch b = core//4, query-block qb = core%4). Each core gets its
batch's x rolled so its 1024 queries are columns 0:1024; attention key/value
order is permutation-invariant so the roll is free. No cross-core communication.

Math (validated in numpy to 6e-8 rel err):
  GroupNorm folded into per-channel affine A, B applied to the weights:
    hn = A*x + B (per channel)
    q  = (wq*A) @ x + (wq@B + bq)
    k-bias drops (softmax shift invariance); v/o biases collapse to
    bo'' = wo@(wv@B + bv) + bo added at the end.
  logitsT[j,i] = sum_ci x[ci,j] * (A[ci] * (wk^T q)[ci,i])   (keys-major layout,
    so softmax reduction is a ones-matmul and no transposes are ever needed)
  P = exp(logitsT/sqrt(C)) unnormalized; o = (wv*A@x) @ P; the division by the
  column sums is applied to the projection output (it commutes with wo@).

All heavy matmuls run as float32r (full PE rate at free dim 512); tiles feeding
the PE are declared float32r so every producer satisfies the BIR rounding rule,
and f32-only consumers (reductions, residual add, bias matmuls) read via bitcast.

Schedule notes (round 2, from perfetto analysis of the 304us v1):
  - DMA issue order: tiny vectors first, then x tiles, then weights — the
    group-selector used by the very first stats matmul otherwise lands last.
  - PE warm-up matmuls on a zero tile keep the HAM clock-gate at 2.4 GHz
    through the prologue so the real matmul stream starts warm.
  - GroupNorm stats: sum(x) on DVE (tensor_reduce, 2x mode) + sum(x^2) on the
    otherwise-idle ACT (Square + accum_out into the q scratch); all Squares
    emitted before all Sqrts so the ACT table loads exactly twice.
  - Chunk epilogue: plain o copies -> projection immediately; 1/s broadcast
    happens in parallel and is applied in the final output DVE op.
"""

import numpy as np

import concourse.bass as bass
import concourse.bacc as bacc
import concourse.tile as tile
from concourse import mybir
from concourse.bass_utils import run_bass_kernel_spmd

F32 = mybir.dt.float32
F32R = mybir.dt.float32r
AF = mybir.ActivationFunctionType
ALU = mybir.AluOpType
AX = mybir.AxisListType

B, C, HH, WW = 2, 512, 64, 64
N = HH * WW          # 4096 pixels
NQ = N // 4          # queries per core
G = 32               # groups
GPT = 8              # groups per 128-channel tile
NT = C // 128        # 4 channel tiles
JT = N // 128        # 32 key tiles
CW = 512             # query chunk width
NCH = NQ // CW       # 2 chunks per core
EPS = 1e-6
SCALE = float(C) ** -0.5
GDIV = 1.0 / 16.0  # st2 carries per-channel means; groups have 16 channels

_CACHE: dict = {}


def _f32(ap):
    return ap.bitcast(F32)


def _build_bass():
    nc = bacc.Bacc("TRN2")

    warm_d = nc.declare_dram_parameter("warm", [128, 128], F32, isOutput=False)
    x_d = nc.declare_dram_parameter("x", [C, N], F32R, isOutput=False)
    wqT_d = nc.declare_dram_parameter("wqT", [C, C], F32R, isOutput=False)
    wk_d = nc.declare_dram_parameter("wk", [C, C], F32R, isOutput=False)
    wvT_d = nc.declare_dram_parameter("wvT", [C, C], F32R, isOutput=False)
    woT_d = nc.declare_dram_parameter("woT", [C, C], F32R, isOutput=False)
    gnw_d = nc.declare_dram_parameter("gnw", [C], F32, isOutput=False)
    gnb_d = nc.declare_dram_parameter("gnb", [C], F32, isOutput=False)
    bq_d = nc.declare_dram_parameter("bq", [C], F32, isOutput=False)
    bv_d = nc.declare_dram_parameter("bv", [C], F32, isOutput=False)
    bo_d = nc.declare_dram_parameter("bo", [C], F32, isOutput=False)
    sel_d = nc.declare_dram_parameter("sel", [128, GPT], F32, isOutput=False)
    selT_d = nc.declare_dram_parameter("selT", [GPT, 128], F32, isOutput=False)
    out_d = nc.declare_dram_parameter("out", [C, NQ], F32, isOutput=True)

    dram = dict(warm=warm_d, x=x_d, wqT=wqT_d, wk=wk_d, wvT=wvT_d, woT=woT_d,
                gnw=gnw_d, gnb=gnb_d, bq=bq_d, bv=bv_d, bo=bo_d,
                sel=sel_d, selT=selT_d, out=out_d)
    with tile.TileContext(nc) as tc, \
         nc.allow_low_precision(reason="float32r tiles are 4-byte fp32 feeding the PE"):
        _emit(tc, {k: v.ap() for k, v in dram.items()})
    nc.compile()
    return nc


def _emit(tc, d):
    nc = tc.nc

    # ---- long-lived pools -------------------------------------------------
    xp = tc.alloc_tile_pool(name="xp", bufs=NT)
    wp = tc.alloc_tile_pool(name="wp", bufs=NT)        # wk, woT (per tag)
    wearly = tc.alloc_tile_pool(name="wear", bufs=NT)  # wqT, wvT
    vecs = tc.alloc_tile_pool(name="vecs", bufs=1)
    qp = tc.alloc_tile_pool(name="qp", bufs=NT)
    vtp = tc.alloc_tile_pool(name="vtp", bufs=JT)

    # ---- DMA in (order matters: small tensors first, x before weights) ----
    warm_sb = vecs.tile([128, 128], F32, tag="warm")
    nc.sync.dma_start(out=warm_sb[:, :], in_=d["warm"])
    sel_sb = vecs.tile([128, GPT], F32, tag="sel")
    nc.sync.dma_start(out=sel_sb[:, :], in_=d["sel"])
    selT_sb = vecs.tile([GPT, 128], F32, tag="selT")
    nc.sync.dma_start(out=selT_sb[:, :], in_=d["selT"])

    def load_vec(name, tag):
        vt = vecs.tile([128, NT], F32, tag=tag)
        nc.sync.dma_start(out=vt[:, :], in_=d[name].rearrange("(t p) -> p t", p=128))
        return vt

    gnw_sb = load_vec("gnw", "gnw")
    gnb_sb = load_vec("gnb", "gnb")
    bqv_sb = load_vec("bq", "bqv")
    bvv_sb = load_vec("bv", "bvv")
    bov_sb = load_vec("bo", "bov")

    x_sb = []
    XSP = 2  # x DMA split factor per tile
    XW = N // XSP
    x_t = d["x"].rearrange("(t p) n -> t p n", p=128)
    for t in range(NT):
        xt = xp.tile([128, N], F32R, tag="x", name=f"xt{t}")
        for hh in range(XSP):
            nc.sync.dma_start(out=xt[:, hh * XW:(hh + 1) * XW],
                              in_=x_t[t][:, hh * XW:(hh + 1) * XW])
        x_sb.append(xt)

    def load_w(pool, name, tag):
        tiles = []
        r = d[name].rearrange("(t p) m -> t p m", p=128)
        for t in range(NT):
            wt = pool.tile([128, C], F32R, tag=tag)
            nc.sync.dma_start(out=wt[:, :], in_=r[t])
            tiles.append(wt)
        return tiles

    wqT_sb = load_w(wearly, "wqT", "wqT")
    wvT_sb = load_w(wearly, "wvT", "wvT")
    wk_sb = load_w(wp, "wk", "wk")
    woT_sb = load_w(wp, "woT", "woT")

    ones32_sb = vecs.tile([128, 128], F32, tag="ones32")
    nc.vector.memset(ones32_sb[:, :], 1.0)
    ones128_sb = vecs.tile([128, 128], F32R, tag="ones128")
    nc.vector.tensor_copy(out=ones128_sb[:, :], in_=ones32_sb[:, :])

    A_sb = vecs.tile([128, NT], F32, tag="A")
    B_sb = vecs.tile([128, NT], F32, tag="B")
    bqp_sb = vecs.tile([128, NT], F32, tag="bqp")
    bvp_sb = vecs.tile([128, NT], F32, tag="bvp")
    bop_sb = vecs.tile([128, NT], F32, tag="bop")

    # q tiles double as ACT scratch for the Square pass during stats
    q_sb = [qp.tile([128, NQ], F32R, tag="q", name=f"q{i}") for i in range(4)]

    # ---- GroupNorm stats → per-channel affine A, B ------------------------
    with tc.tile_pool(name="stp", bufs=4) as stp, \
         tc.tile_pool(name="pssm", bufs=2, space="PSUM") as ps_sm:
        nwarm = [0]

        def emit_warm(n):
            for _ in range(n):
                wt = ps_sm.tile([128, 128], F32, tag="warm", name=f"wm{nwarm[0]}")
                nwarm[0] += 1
                nc.tensor.matmul(out=wt[:, :], lhsT=warm_sb[:, 0:128],
                                 rhs=warm_sb[:, :], start=True, stop=True)

        emit_warm(12)
        gps_t = []
        for t in range(NT):
            st2 = stp.tile([128, 2], F32, tag="st2", name=f"st2_{t}")
            if t < NT - 1:
                # DVE bn_stats per half as the DMA lands
                st = stp.tile([128, 8, 6], F32, tag="bnst", name=f"bnst{t}")
                xr = _f32(x_sb[t][:, :]).rearrange("p (s n) -> p s n", s=8)
                for s in range(8):
                    nc.vector.bn_stats(out=st[:, s, :], in_=xr[:, s, :])
                mv = stp.tile([128, 2], F32, tag="mv", name=f"mv{t}")
                nc.vector.bn_aggr(out=mv[:, :], in_=st[:, :, :])
                nc.vector.tensor_copy(out=st2[:, 0:1], in_=mv[:, 0:1])
                nc.vector.tensor_mul(out=st2[:, 1:2], in0=mv[:, 0:1], in1=mv[:, 0:1])
                nc.vector.tensor_add(out=st2[:, 1:2], in0=st2[:, 1:2], in1=mv[:, 1:2])
            else:
                # last tile: sum(x^2) on ACT (Square+accum into q scratch),
                # sum(x) on DVE — the two engines run in parallel
                sq4 = stp.tile([128, NT], F32, tag="sq4", name=f"sq4_{t}")
                for k in range(4):
                    nc.scalar.activation(out=q_sb[k][:, :],
                                         in_=x_sb[t][:, k * NQ:(k + 1) * NQ],
                                         func=AF.Square, bias=0.0, scale=1.0,
                                         accum_out=sq4[:, k:k + 1])
                nc.vector.tensor_reduce(out=st2[:, 0:1], in_=_f32(x_sb[t][:, :]),
                                        axis=AX.X, op=ALU.add)
                nc.vector.tensor_scalar_mul(out=st2[:, 0:1], in0=st2[:, 0:1],
                                            scalar1=1.0 / N)
                nc.vector.tensor_reduce(out=st2[:, 1:2], in_=sq4[:, :],
                                        axis=AX.X, op=ALU.add)
                nc.vector.tensor_scalar_mul(out=st2[:, 1:2], in0=st2[:, 1:2],
                                            scalar1=1.0 / N)
            gps = ps_sm.tile([GPT, 2], F32, tag="gps", name=f"gps{t}")
            nc.tensor.matmul(out=gps[:, :], lhsT=sel_sb[:, :], rhs=st2[:, :],
                             start=True, stop=True)
            gps_t.append(gps)
            emit_warm((10, 10, 6, 0)[t])

        # group mean / rstd; all DVE preps first, then batched ACT Sqrts
        grp_t = []
        for t in range(NT):
            grp = stp.tile([GPT, 2], F32, tag="grp", name=f"grp{t}")
            nc.vector.tensor_scalar_mul(out=grp[:, :], in0=gps_t[t][:, :], scalar1=GDIV)
            gtmp = stp.tile([GPT, 1], F32, tag="gtmp", name=f"gtmp{t}")
            nc.vector.tensor_mul(out=gtmp[:, :], in0=grp[:, 0:1], in1=grp[:, 0:1])
            nc.vector.tensor_sub(out=grp[:, 1:2], in0=grp[:, 1:2], in1=gtmp[:, :])
            nc.vector.tensor_scalar_add(out=grp[:, 1:2], in0=grp[:, 1:2], scalar1=EPS)
            grp_t.append(grp)
        for t in range(NT):
            nc.scalar.activation(out=grp_t[t][:, 1:2], in_=grp_t[t][:, 1:2],
                                 func=AF.Sqrt, bias=0.0, scale=1.0)
        emit_warm(2)
        for t in range(NT):
            nc.vector.reciprocal(out=grp_t[t][:, 1:2], in_=grp_t[t][:, 1:2])
            mrp = ps_sm.tile([128, 2], F32, tag="sm", name=f"mrp{t}")
            nc.tensor.matmul(out=mrp[:, :], lhsT=selT_sb[:, :], rhs=grp_t[t][:, :],
                             start=True, stop=True)
            tcol = slice(t, t + 1)
            nc.vector.tensor_mul(out=A_sb[:, tcol], in0=gnw_sb[:, tcol], in1=mrp[:, 1:2])
            nc.vector.tensor_mul(out=B_sb[:, tcol], in0=mrp[:, 0:1], in1=A_sb[:, tcol])
            nc.vector.tensor_sub(out=B_sb[:, tcol], in0=gnb_sb[:, tcol], in1=B_sb[:, tcol])

    ps_mm = tc.alloc_tile_pool(name="psmm", bufs=3, space="PSUM")

    # ---- folded biases (need un-scaled wqT/wvT, so run before scaling) ----
    for ot in range(4):
        ocol = slice(ot, ot + 1)
        bps = ps_mm.tile([128, 1], F32, tag="mm", name=f"bq{ot}")
        for ci in range(NT):
            nc.tensor.matmul(out=bps[:, :],
                             lhsT=_f32(wqT_sb[ci][:, ot * 128:(ot + 1) * 128]),
                             rhs=B_sb[:, ci:ci + 1],
                             start=(ci == 0), stop=(ci == NT - 1))
        nc.vector.tensor_add(out=bqp_sb[:, ocol], in0=bps[:, :], in1=bqv_sb[:, ocol])
    for ot in range(4):
        ocol = slice(ot, ot + 1)
        bps2 = ps_mm.tile([128, 1], F32, tag="mm", name=f"bv{ot}")
        for ci in range(NT):
            nc.tensor.matmul(out=bps2[:, :],
                             lhsT=_f32(wvT_sb[ci][:, ot * 128:(ot + 1) * 128]),
                             rhs=B_sb[:, ci:ci + 1],
                             start=(ci == 0), stop=(ci == NT - 1))
        nc.vector.tensor_add(out=bvp_sb[:, ocol], in0=bps2[:, :], in1=bvv_sb[:, ocol])

    # ---- scale wq^T / wv^T rows by A, then q = wqA @ x[:, 0:NQ] + bq' -----
    for t in range(NT):
        nc.vector.tensor_scalar_mul(out=wqT_sb[t][:, :], in0=_f32(wqT_sb[t][:, :]),
                                    scalar1=A_sb[:, t:t + 1])
    for t in range(NT):
        nc.vector.tensor_scalar_mul(out=wvT_sb[t][:, :], in0=_f32(wvT_sb[t][:, :]),
                                    scalar1=A_sb[:, t:t + 1])
    for ot in range(4):
        for ch in range(NCH):
            csl = slice(ch * CW, (ch + 1) * CW)
            qps = ps_mm.tile([128, CW], F32, tag="mm")
            for ci in range(NT):
                nc.tensor.matmul(out=qps[:, :],
                                 lhsT=wqT_sb[ci][:, ot * 128:(ot + 1) * 128],
                                 rhs=x_sb[ci][:, csl],
                                 start=(ci == 0), stop=(ci == NT - 1))
            nc.vector.tensor_scalar_add(out=q_sb[ot][:, csl], in0=qps[:, :],
                                        scalar1=bqp_sb[:, ot:ot + 1])

    ps_o = tc.alloc_tile_pool(name="pso", bufs=4, space="PSUM")

    # ---- vT[j, c] = ((wv*A) @ x)^T ----------------------------------------
    vt_sb = []
    for jt in range(JT):
        jsl = slice(jt * 128, (jt + 1) * 128)
        vps = ps_mm.tile([128, C], F32, tag="mm")
        for ci in range(NT):
            nc.tensor.matmul(out=vps[:, :], lhsT=x_sb[ci][:, jsl],
                             rhs=wvT_sb[ci][:, :],
                             start=(ci == 0), stop=(ci == NT - 1))
        vt = vtp.tile([128, C], F32R, tag="vt")
        nc.vector.tensor_copy(out=vt[:, :], in_=vps[:, :])
        vt_sb.append(vt)

    # ---- bo'' = wo@bv' + bo (emitted here so it never waits on the late woT DMA)
    for ot in range(4):
        ocol = slice(ot, ot + 1)
        bps3 = ps_mm.tile([128, 1], F32, tag="mm", name=f"bo{ot}")
        for ci in range(NT):
            nc.tensor.matmul(out=bps3[:, :],
                             lhsT=_f32(woT_sb[ci][:, ot * 128:(ot + 1) * 128]),
                             rhs=bvp_sb[:, ci:ci + 1],
                             start=(ci == 0), stop=(ci == NT - 1))
        nc.vector.tensor_add(out=bop_sb[:, ocol], in0=bps3[:, :], in1=bov_sb[:, ocol])

    # ---- attention chunks -------------------------------------------------
    qkp = tc.alloc_tile_pool(name="qkp", bufs=NT)
    pp = tc.alloc_tile_pool(name="pp", bufs=2)
    osb = tc.alloc_tile_pool(name="osb", bufs=4)
    outp = tc.alloc_tile_pool(name="outp", bufs=2)
    smsb = tc.alloc_tile_pool(name="smsb", bufs=1)

    for ch in range(NCH):
        csl = slice(ch * CW, (ch + 1) * CW)
        # qk[ci, i] = A[ci] * (wk^T q)[ci, i]
        qk_sb = []
        for ci in range(NT):
            kps = ps_mm.tile([128, CW], F32, tag="mm")
            for ot in range(4):
                nc.tensor.matmul(out=kps[:, :],
                                 lhsT=wk_sb[ot][:, ci * 128:(ci + 1) * 128],
                                 rhs=q_sb[ot][:, csl],
                                 start=(ot == 0), stop=(ot == NT - 1))
            qk = qkp.tile([128, CW], F32R, tag="qk")
            nc.vector.tensor_scalar_mul(out=qk[:, :], in0=kps[:, :],
                                        scalar1=A_sb[:, ci:ci + 1])
            qk_sb.append(qk)

        o_ps = [ps_o.tile([128, CW], F32, tag="o", name=f"o{ch}_{i}") for i in range(4)]
        sacc = smsb.tile([128, CW], F32R, tag="sacc", name=f"sacc{ch}")
        for jt in range(JT):
            jsl = slice(jt * 128, (jt + 1) * 128)
            lps = ps_mm.tile([128, CW], F32, tag="mm")
            for ci in range(NT):
                nc.tensor.matmul(out=lps[:, :], lhsT=x_sb[ci][:, jsl],
                                 rhs=qk_sb[ci][:, :],
                                 start=(ci == 0), stop=(ci == NT - 1))
            P = pp.tile([128, CW], F32R, tag="P")
            nc.scalar.activation(out=P[:, :], in_=lps[:, :], func=AF.Exp,
                                 bias=0.0, scale=SCALE)
            for co in range(4):
                nc.tensor.matmul(out=o_ps[co][:, :],
                                 lhsT=vt_sb[jt][:, co * 128:(co + 1) * 128],
                                 rhs=P[:, :],
                                 start=(jt == 0), stop=(jt == JT - 1),
                                 skip_group_check=True)
            if jt == 0:
                nc.vector.tensor_copy(out=sacc[:, :], in_=_f32(P[:, :]))
            else:
                nc.vector.tensor_add(out=sacc[:, :], in0=_f32(sacc[:, :]),
                                     in1=_f32(P[:, :]))

        # epilogue: plain o copies -> project immediately; 1/s broadcast in
        # parallel; normalize + bias + residual fused in the final DVE ops.
        last = ch == NCH - 1
        if last:
            # tail chunk: normalize during the PSUM->SBUF copy so the final
            # DVE chain is 2 ops; costs a small PE stall waiting for 1/s
            rbp = ps_mm.tile([128, CW], F32, tag="mm")
            nc.tensor.matmul(out=rbp[:, :], lhsT=ones128_sb[:, :], rhs=sacc[:, :],
                             start=True, stop=True)
            rsb = smsb.tile([128, CW], F32, tag="rsb")
            nc.vector.reciprocal_approx_fast(out=rsb[:, :], in_=rbp[:, :])
        o_sb = []
        for co in range(4):
            ot_ = osb.tile([128, CW], F32R, tag="osb")
            if last:
                nc.vector.tensor_mul(out=ot_[:, :], in0=o_ps[co][:, :], in1=rsb[:, :])
            else:
                nc.vector.tensor_copy(out=ot_[:, :], in_=o_ps[co][:, :])
            o_sb.append(ot_)
        prp_t = []
        for co in range(4):
            prp = ps_o.tile([128, CW], F32, tag="o", name=f"pr{ch}_{co}")
            for c in range(NT):
                nc.tensor.matmul(out=prp[:, :],
                                 lhsT=woT_sb[c][:, co * 128:(co + 1) * 128],
                                 rhs=o_sb[c][:, :],
                                 start=(c == 0), stop=(c == NT - 1))
            prp_t.append(prp)
        if not last:
            rbp = ps_mm.tile([128, CW], F32, tag="mm")
            nc.tensor.matmul(out=rbp[:, :], lhsT=ones128_sb[:, :], rhs=sacc[:, :],
                             start=True, stop=True)
            rsb = smsb.tile([128, CW], F32, tag="rsb")
            nc.vector.reciprocal_approx_fast(out=rsb[:, :], in_=rbp[:, :])
        for co in range(4):
            ou = outp.tile([128, CW], F32, tag="out")
            if last:
                nc.vector.tensor_scalar_add(out=ou[:, :], in0=prp_t[co][:, :],
                                            scalar1=bop_sb[:, co:co + 1])
            else:
                nc.vector.tensor_mul(out=ou[:, :], in0=prp_t[co][:, :], in1=rsb[:, :])
                nc.vector.tensor_scalar_add(out=ou[:, :], in0=ou[:, :],
                                            scalar1=bop_sb[:, co:co + 1])
            nc.vector.tensor_add(out=ou[:, :], in0=ou[:, :],
                                 in1=_f32(x_sb[co][:, csl]))
            nc.sync.dma_start(out=d["out"][co * 128:(co + 1) * 128, csl], in_=ou[:, :])

    for p in (smsb, outp, osb, pp, qkp, ps_o, ps_mm, vtp, qp, vecs,
              wearly, wp, xp):
        p.release()


def _sel_consts():
    sel = np.zeros((128, GPT), np.float32)
    for p in range(128):
        sel[p, p // 16] = 1.0
    return sel, np.ascontiguousarray(sel.T)


def kernel(x, gn_w, gn_b, wq, bq, wk, bk, wv, bv, wo, bo):
    del bk  # exactly cancelled by softmax shift invariance
    if "nc" not in _CACHE:
        _CACHE["nc"] = _build_bass()
    nc = _CACHE["nc"]

    x = np.ascontiguousarray(np.asarray(x, np.float32)).reshape(B, C, N)
    wqT = np.ascontiguousarray(np.asarray(wq, np.float32).T)
    wkn = np.ascontiguousarray(np.asarray(wk, np.float32))
    wvT = np.ascontiguousarray(np.asarray(wv, np.float32).T)
    woT = np.ascontiguousarray(np.asarray(wo, np.float32).T)
    vecs = {n: np.ascontiguousarray(np.asarray(v, np.float32))
            for n, v in (("gnw", gn_w), ("gnb", gn_b), ("bq", bq), ("bv", bv),
                         ("bo", bo))}
    sel, selT = _sel_consts()
    warm = np.zeros((128, 128), np.float32)

    in_maps = []
    for core in range(8):
        b, qb = core // 4, core % 4
        xb = np.ascontiguousarray(np.roll(x[b], -qb * NQ, axis=1))
        in_maps.append({"x": xb, "wqT": wqT, "wk": wkn, "wvT": wvT, "woT": woT,
                        "sel": sel, "selT": selT, "warm": warm, **vecs})

    _CACHE["last_in_maps"] = in_maps
    res = run_bass_kernel_spmd(nc, in_maps, list(range(8))).results
    out = np.empty((B, C, N), np.float32)
    for core in range(8):
        b, qb = core // 4, core % 4
        out[b][:, qb * NQ:(qb + 1) * NQ] = res[core]["out"]
    return out.reshape(B, C, HH, WW)



# revision 7
# speedup vs baseline: 1.1597x; 1.1597x over previous
"""AttnBlock (GroupNorm + single-head self-attention + proj + residual) on 8 trn2 cores.

Sharding: core = (batch b = core//4, query-block qb = core%4). Each core gets its
batch's x rolled so its 1024 queries are columns 0:1024; attention key/value
order is permutation-invariant so the roll is free. No cross-core communication.

Math (v3, bf16 PE path):
  GroupNorm folded into per-channel affine A, B: hn = A*x + B.
  Logits fold ("M-trick"): logits[j,i] = x[:,j]^T A (wk^T wq) A x[:,i] + gamma[j]
    MTraw[cq,ck] = sum_o wq[o,cq] wk[o,ck]   -- needs only weights, computed
      during the x DMA (real work in the prologue instead of warm-up matmuls)
    MT1 = A_q (row) * MTraw;  qk_ps = MT1^T-blocks @ x;  qk = A_k*qk_ps + A_k*c0
    c0[ck] = sum_cq MTraw[cq,ck] B[cq] + (wk^T bq)[ck]  (k-bias itself drops by
      softmax shift invariance; the q-bias survives as this per-key offset)
  P = exp(logits/sqrt(C)) unnormalized; o = (wv*A@x) @ P; normalization by the
  column sums commutes with wo@ and is applied at the projection output.
  v/o biases collapse to bo'' = wo@(wv@B + bv) + bo, pre-added to the residual.

Precision: all PE inputs bf16 (1 cycle/row at any free size, 53ns LDWEIGHTS);
all accumulation f32 in PSUM; softmax P-sum accumulated f32 on DVE; residual x
kept f32 (separate late DMA) so the final add is exact. Measured rel err vs the
f32 reference must stay < 2e-2 (fp32r version measured 1.05e-4).
"""

import numpy as np
import ml_dtypes

import concourse.bass as bass
import concourse.bacc as bacc
import concourse.tile as tile
from concourse import mybir
from concourse.bass_utils import run_bass_kernel_spmd

F32 = mybir.dt.float32
F32R = mybir.dt.float32r
BF16 = mybir.dt.bfloat16
AF = mybir.ActivationFunctionType
ALU = mybir.AluOpType
AX = mybir.AxisListType

B, C, HH, WW = 2, 512, 64, 64
N = HH * WW          # 4096 pixels
NQ = N // 4          # queries per core
G = 32               # groups
GPT = 8              # groups per 128-channel tile
NT = C // 128        # 4 channel tiles
JT = N // 128        # 32 key tiles
CW = 512             # query chunk width
NCH = NQ // CW       # 2 chunks per core
EPS = 1e-6
SCALE = float(C) ** -0.5
GDIV = 1.0 / 16.0  # st2 carries per-channel means; groups have 16 channels

_CACHE: dict = {}


def _f32(ap):
    return ap.bitcast(F32)


def _build_bass():
    nc = bacc.Bacc("TRN2")

    wq_d = nc.declare_dram_parameter("wq", [C, C], BF16, isOutput=False)
    wk_d = nc.declare_dram_parameter("wk", [C, C], BF16, isOutput=False)
    x_d = nc.declare_dram_parameter("x", [C, N], BF16, isOutput=False)
    wvT_d = nc.declare_dram_parameter("wvT", [C, C], BF16, isOutput=False)
    woT_d = nc.declare_dram_parameter("woT", [C, C], BF16, isOutput=False)
    xr_d = nc.declare_dram_parameter("xr", [C, NQ], F32, isOutput=False)
    gnw_d = nc.declare_dram_parameter("gnw", [C], F32, isOutput=False)
    gnb_d = nc.declare_dram_parameter("gnb", [C], F32, isOutput=False)
    bq_d = nc.declare_dram_parameter("bq", [C], BF16, isOutput=False)
    bv_d = nc.declare_dram_parameter("bv", [C], F32, isOutput=False)
    bo_d = nc.declare_dram_parameter("bo", [C], F32, isOutput=False)
    sel_d = nc.declare_dram_parameter("sel", [128, GPT], F32, isOutput=False)
    selT_d = nc.declare_dram_parameter("selT", [GPT, 128], F32, isOutput=False)
    out_d = nc.declare_dram_parameter("out", [C, NQ], F32, isOutput=True)

    dram = dict(wq=wq_d, wk=wk_d, x=x_d, wvT=wvT_d, woT=woT_d, xr=xr_d,
                gnw=gnw_d, gnb=gnb_d, bq=bq_d, bv=bv_d, bo=bo_d,
                sel=sel_d, selT=selT_d, out=out_d)
    with tile.TileContext(nc) as tc, \
         nc.allow_low_precision(reason="bf16 PE inputs with f32 accumulation"):
        _emit(tc, {k: v.ap() for k, v in dram.items()})
    nc.compile()
    return nc


def _emit(tc, d):
    nc = tc.nc

    # ---- long-lived pools -------------------------------------------------
    xp = tc.alloc_tile_pool(name="xp", bufs=NT)
    rp = tc.alloc_tile_pool(name="rp", bufs=NT)
    wqk = tc.alloc_tile_pool(name="wqk", bufs=2 * NT)   # wq, wk
    wvo = tc.alloc_tile_pool(name="wvo", bufs=2 * NT)   # wvT, woT
    mp = tc.alloc_tile_pool(name="mp", bufs=NT)         # MTraw -> MT1 in place
    vecs = tc.alloc_tile_pool(name="vecs", bufs=1)
    qkp = tc.alloc_tile_pool(name="qkp", bufs=2 * NT)   # both chunks
    vtp = tc.alloc_tile_pool(name="vtp", bufs=JT)

    # ---- DMA in (issue order == effective arrival order) ------------------
    sel_sb = vecs.tile([128, GPT], F32, tag="sel")
    nc.sync.dma_start(out=sel_sb[:, :], in_=d["sel"])
    selT_sb = vecs.tile([GPT, 128], F32, tag="selT")
    nc.sync.dma_start(out=selT_sb[:, :], in_=d["selT"])

    def load_vec(name, tag, dt=F32):
        vt = vecs.tile([128, NT], dt, tag=tag)
        nc.sync.dma_start(out=vt[:, :], in_=d[name].rearrange("(t p) -> p t", p=128))
        return vt

    gnw_sb = load_vec("gnw", "gnw")
    gnb_sb = load_vec("gnb", "gnb")
    bqv_sb = load_vec("bq", "bqv", BF16)
    bvv_sb = load_vec("bv", "bvv")
    bov_sb = load_vec("bo", "bov")

    def load_w(pool, name, tag):
        tiles = []
        r = d[name].rearrange("(t p) m -> t p m", p=128)
        for t in range(NT):
            wt = pool.tile([128, C], BF16, tag=tag)
            nc.sync.dma_start(out=wt[:, :], in_=r[t])
            tiles.append(wt)
        return tiles

    wk_sb = load_w(wqk, "wk", "wk")
    wq_sb = load_w(wqk, "wq", "wq")

    # x: h-major halves so every tile's first half lands early for bn_stats
    XSP = 2
    XW = N // XSP
    x_t = d["x"].rearrange("(t p) n -> t p n", p=128)
    x_sb = [xp.tile([128, N], BF16, tag="x", name=f"xt{t}") for t in range(NT)]
    for hh in range(XSP):
        for t in range(NT):
            nc.sync.dma_start(out=x_sb[t][:, hh * XW:(hh + 1) * XW],
                              in_=x_t[t][:, hh * XW:(hh + 1) * XW])

    wvT_sb = load_w(wvo, "wvT", "wvT")
    woT_sb = load_w(wvo, "woT", "woT")

    # residual (f32) -- only needed at chunk epilogues, lands last
    xr_t = d["xr"].rearrange("(t p) n -> t p n", p=128)
    xr_sb = []
    for t in range(NT):
        rt = rp.tile([128, NQ], F32, tag="xr", name=f"xr{t}")
        nc.sync.dma_start(out=rt[:, :], in_=xr_t[t])
        xr_sb.append(rt)

    # ---- SBUF constants ---------------------------------------------------
    warm_sb = vecs.tile([128, CW], BF16, tag="warm")
    nc.vector.memset(warm_sb[:, :], 0.0)
    ones32_sb = vecs.tile([128, 128], F32, tag="ones32")
    nc.vector.memset(ones32_sb[:, :], 1.0)
    ones128_sb = vecs.tile([128, 128], F32R, tag="ones128")
    nc.vector.tensor_copy(out=ones128_sb[:, :], in_=ones32_sb[:, :])

    A_sb = vecs.tile([128, NT], F32, tag="A")
    B_sb = vecs.tile([128, NT], F32, tag="B")
    Bp_sb = vecs.tile([128, NT], BF16, tag="Bp")    # B/A, bf16 for c0 rhs
    Bb_sb = vecs.tile([128, NT], BF16, tag="Bb")    # B, bf16 for bv' rhs
    bvpb_sb = vecs.tile([128, NT], BF16, tag="bvpb")  # bv' bf16 for bo fold
    c0A_sb = vecs.tile([128, NT], F32, tag="c0A")
    bvp_sb = vecs.tile([128, NT], F32, tag="bvp")
    bop_sb = vecs.tile([128, NT], F32, tag="bop")

    # ---- prologue: warm-up + M_raw + GroupNorm stats ----------------------
    with tc.tile_pool(name="stp", bufs=4) as stp, \
         tc.tile_pool(name="pspro", bufs=2, space="PSUM") as ps_pro:
        nwarm = [0]

        def emit_warm(n):
            for _ in range(n):
                wt = ps_pro.tile([128, CW], F32, tag="pro", name=f"wm{nwarm[0]}")
                nwarm[0] += 1
                nc.tensor.matmul(out=wt[:, :], lhsT=warm_sb[:, 0:128],
                                 rhs=warm_sb[:, :], start=True, stop=True)

        emit_warm(6)

        # MTraw[cq, ck] = sum_o wq[o, cq] wk[o, ck]  (weights only, no x/A dep)
        mt_sb = []
        for cq in range(NT):
            mps = ps_pro.tile([128, C], F32, tag="pro", name=f"mps{cq}")
            for ot in range(NT):
                nc.tensor.matmul(out=mps[:, :],
                                 lhsT=wq_sb[ot][:, cq * 128:(cq + 1) * 128],
                                 rhs=wk_sb[ot][:, :],
                                 start=(ot == 0), stop=(ot == NT - 1))
            mt = mp.tile([128, C], BF16, tag="mt", name=f"mt{cq}")
            nc.vector.tensor_copy(out=mt[:, :], in_=mps[:, :])
            mt_sb.append(mt)

        # bn_stats per half-tile as the DMA lands; st2 = [mean, E[x^2]]
        gps_t = []
        st_t = []
        for t in range(NT):
            st = stp.tile([128, 8, 6], F32, tag="bnst", name=f"bnst{t}")
            st_t.append(st)
        for hh in range(XSP):
            for t in range(NT):
                xr_ = x_sb[t][:, hh * XW:(hh + 1) * XW].rearrange(
                    "p (s n) -> p s n", s=4)
                for s in range(4):
                    nc.vector.bn_stats(out=st_t[t][:, hh * 4 + s, :],
                                       in_=xr_[:, s, :])
        emit_warm(8)
        for t in range(NT):
            st2 = stp.tile([128, 2], F32, tag="st2", name=f"st2_{t}")
            mv = stp.tile([128, 2], F32, tag="mv", name=f"mv{t}")
            nc.vector.bn_aggr(out=mv[:, :], in_=st_t[t][:, :, :])
            nc.vector.tensor_copy(out=st2[:, 0:1], in_=mv[:, 0:1])
            nc.vector.tensor_mul(out=st2[:, 1:2], in0=mv[:, 0:1], in1=mv[:, 0:1])
            nc.vector.tensor_add(out=st2[:, 1:2], in0=st2[:, 1:2], in1=mv[:, 1:2])
            gps = ps_pro.tile([GPT, 2], F32, tag="gps", name=f"gps{t}")
            nc.tensor.matmul(out=gps[:, :], lhsT=sel_sb[:, :], rhs=st2[:, :],
                             start=True, stop=True)
            gps_t.append(gps)
            emit_warm((3, 3, 3, 0)[t])

        # group mean / rstd; DVE preps first, then batched ACT Sqrts
        grp_t = []
        for t in range(NT):
            grp = stp.tile([GPT, 2], F32, tag="grp", name=f"grp{t}")
            nc.vector.tensor_scalar_mul(out=grp[:, :], in0=gps_t[t][:, :], scalar1=GDIV)
            gtmp = stp.tile([GPT, 1], F32, tag="gtmp", name=f"gtmp{t}")
            nc.vector.tensor_mul(out=gtmp[:, :], in0=grp[:, 0:1], in1=grp[:, 0:1])
            nc.vector.tensor_sub(out=grp[:, 1:2], in0=grp[:, 1:2], in1=gtmp[:, :])
            nc.vector.tensor_scalar_add(out=grp[:, 1:2], in0=grp[:, 1:2], scalar1=EPS)
            grp_t.append(grp)
        for t in range(NT):
            nc.scalar.activation(out=grp_t[t][:, 1:2], in_=grp_t[t][:, 1:2],
                                 func=AF.Sqrt, bias=0.0, scale=1.0)
        for t in range(NT):
            nc.vector.reciprocal(out=grp_t[t][:, 1:2], in_=grp_t[t][:, 1:2])
            mrp = ps_pro.tile([128, 2], F32, tag="sm", name=f"mrp{t}")
            nc.tensor.matmul(out=mrp[:, :], lhsT=selT_sb[:, :], rhs=grp_t[t][:, :],
                             start=True, stop=True)
            tcol = slice(t, t + 1)
            nc.vector.tensor_mul(out=A_sb[:, tcol], in0=gnw_sb[:, tcol], in1=mrp[:, 1:2])
            nc.vector.tensor_mul(out=B_sb[:, tcol], in0=mrp[:, 0:1], in1=A_sb[:, tcol])
            nc.vector.tensor_sub(out=B_sb[:, tcol], in0=gnb_sb[:, tcol], in1=B_sb[:, tcol])
            # Bp = B / A (bf16) for the c0 fold through A-scaled MT1
            rA = stp.tile([128, 1], F32, tag="rA", name=f"rA{t}")
            nc.vector.reciprocal(out=rA[:, :], in_=A_sb[:, tcol])
            nc.vector.tensor_mul(out=Bp_sb[:, tcol], in0=B_sb[:, tcol], in1=rA[:, :])
            nc.vector.tensor_copy(out=Bb_sb[:, tcol], in_=B_sb[:, tcol])

        # MT1 = A_q (per-partition) * MTraw, in place
        for cq in range(NT):
            nc.vector.tensor_scalar_mul(out=mt_sb[cq][:, :], in0=mt_sb[cq][:, :],
                                        scalar1=A_sb[:, cq:cq + 1])

    ps_mm = tc.alloc_tile_pool(name="psmm", bufs=3, space="PSUM")

    # ---- c0A[ck] = A_k * (sum_cq MTraw[cq,ck] B[cq] + wk^T bq) ------------
    for ck in range(NT):
        cps = ps_mm.tile([128, 1], F32, tag="mm", name=f"c0{ck}")
        for cq in range(NT):
            nc.tensor.matmul(out=cps[:, :],
                             lhsT=mt_sb[cq][:, ck * 128:(ck + 1) * 128],
                             rhs=Bp_sb[:, cq:cq + 1],
                             start=(cq == 0), stop=False)
        for ot in range(NT):
            nc.tensor.matmul(out=cps[:, :],
                             lhsT=wk_sb[ot][:, ck * 128:(ck + 1) * 128],
                             rhs=bqv_sb[:, ot:ot + 1],
                             start=False, stop=(ot == NT - 1))
        nc.vector.tensor_mul(out=c0A_sb[:, ck:ck + 1], in0=cps[:, :],
                             in1=A_sb[:, ck:ck + 1])

    # ---- qk for both chunks: qk = A_k * (MT1^T-blocks @ x) + c0A ----------
    qk_sb = [[None] * NT for _ in range(NCH)]
    for ch in range(NCH):
        csl = slice(ch * CW, (ch + 1) * CW)
        for ck in range(NT):
            kps = ps_mm.tile([128, CW], F32, tag="mm")
            for cq in range(NT):
                nc.tensor.matmul(out=kps[:, :],
                                 lhsT=mt_sb[cq][:, ck * 128:(ck + 1) * 128],
                                 rhs=x_sb[cq][:, csl],
                                 start=(cq == 0), stop=(cq == NT - 1))
            qk = qkp.tile([128, CW], BF16, tag="qk")
            nc.vector.tensor_scalar(out=qk[:, :], in0=kps[:, :],
                                    scalar1=A_sb[:, ck:ck + 1],
                                    scalar2=c0A_sb[:, ck:ck + 1],
                                    op0=ALU.mult, op1=ALU.add)
            qk_sb[ch][ck] = qk

    # ---- folded v bias: bv' = wv@B + bv (needs unscaled wvT) --------------
    for ot in range(NT):
        ocol = slice(ot, ot + 1)
        bps2 = ps_mm.tile([128, 1], F32, tag="mm", name=f"bv{ot}")
        for ci in range(NT):
            nc.tensor.matmul(out=bps2[:, :],
                             lhsT=wvT_sb[ci][:, ot * 128:(ot + 1) * 128],
                             rhs=Bb_sb[:, ci:ci + 1],
                             start=(ci == 0), stop=(ci == NT - 1))
        nc.vector.tensor_add(out=bvp_sb[:, ocol], in0=bps2[:, :], in1=bvv_sb[:, ocol])
        nc.vector.tensor_copy(out=bvpb_sb[:, ocol], in_=bvp_sb[:, ocol])

    # ---- scale wvT rows by A for the vT matmuls (after the bias fold) -----
    for ci in range(NT):
        nc.vector.tensor_scalar_mul(out=wvT_sb[ci][:, :], in0=wvT_sb[ci][:, :],
                                    scalar1=A_sb[:, ci:ci + 1])

    ps_o = tc.alloc_tile_pool(name="pso", bufs=4, space="PSUM")

    # ---- vT[j, c] = ((wv*A) @ x)^T ----------------------------------------
    vt_sb = []
    for jt in range(JT):
        jsl = slice(jt * 128, (jt + 1) * 128)
        vps = ps_mm.tile([128, C], F32, tag="mm")
        for ci in range(NT):
            nc.tensor.matmul(out=vps[:, :], lhsT=x_sb[ci][:, jsl],
                             rhs=wvT_sb[ci][:, :],
                             start=(ci == 0), stop=(ci == NT - 1))
        vt = vtp.tile([128, C], BF16, tag="vt")
        nc.vector.tensor_copy(out=vt[:, :], in_=vps[:, :])
        vt_sb.append(vt)
        if jt == 1:
            # bo'' = wo@bv' + bo, off the critical path once woT has landed
            for ot2 in range(NT):
                oc2 = slice(ot2, ot2 + 1)
                bps3 = ps_mm.tile([128, 1], F32, tag="mm", name=f"bo{ot2}")
                for ci2 in range(NT):
                    nc.tensor.matmul(out=bps3[:, :],
                                     lhsT=woT_sb[ci2][:, ot2 * 128:(ot2 + 1) * 128],
                                     rhs=bvpb_sb[:, ci2:ci2 + 1],
                                     start=(ci2 == 0), stop=(ci2 == NT - 1))
                nc.vector.tensor_add(out=bop_sb[:, oc2], in0=bps3[:, :],
                                     in1=bov_sb[:, oc2])
            # residual' = x_resid + bo'' (full NQ width, ahead of the tails)
            for co2 in range(NT):
                nc.vector.tensor_scalar_add(out=xr_sb[co2][:, :],
                                            in0=xr_sb[co2][:, :],
                                            scalar1=bop_sb[:, co2:co2 + 1])

    # ---- attention chunks -------------------------------------------------
    pp = tc.alloc_tile_pool(name="pp", bufs=3)
    osb = tc.alloc_tile_pool(name="osb", bufs=4)
    outp = tc.alloc_tile_pool(name="outp", bufs=2)
    smsb = tc.alloc_tile_pool(name="smsb", bufs=1)

    for ch in range(NCH):
        csl = slice(ch * CW, (ch + 1) * CW)
        o_ps = [ps_o.tile([128, CW], F32, tag="o", name=f"o{ch}_{i}") for i in range(4)]
        sacc = smsb.tile([128, CW], F32R, tag="sacc", name=f"sacc{ch}")
        for jt in range(JT):
            jsl = slice(jt * 128, (jt + 1) * 128)
            lps = ps_mm.tile([128, CW], F32, tag="mm")
            for ci in range(NT):
                nc.tensor.matmul(out=lps[:, :], lhsT=x_sb[ci][:, jsl],
                                 rhs=qk_sb[ch][ci][:, :],
                                 start=(ci == 0), stop=(ci == NT - 1))
            P = pp.tile([128, CW], BF16, tag="P")
            nc.scalar.activation(out=P[:, :], in_=lps[:, :], func=AF.Exp,
                                 bias=0.0, scale=SCALE)
            for co in range(4):
                nc.tensor.matmul(out=o_ps[co][:, :],
                                 lhsT=vt_sb[jt][:, co * 128:(co + 1) * 128],
                                 rhs=P[:, :],
                                 start=(jt == 0), stop=(jt == JT - 1),
                                 skip_group_check=True)
            if jt == 0:
                nc.vector.tensor_copy(out=sacc[:, :], in_=P[:, :])
            else:
                nc.vector.tensor_add(out=sacc[:, :], in0=_f32(sacc[:, :]),
                                     in1=P[:, :])

        # epilogue: 1/sums; last chunk normalizes during the o copy so the
        # final DVE chain is one op (residual' already carries bo'').
        last = ch == NCH - 1
        rbp = ps_mm.tile([128, CW], F32, tag="mm")
        nc.tensor.matmul(out=rbp[:, :], lhsT=ones128_sb[:, :], rhs=sacc[:, :],
                         start=True, stop=True)
        rsb = smsb.tile([128, CW], F32, tag="rsb")
        nc.vector.reciprocal_approx_fast(out=rsb[:, :], in_=rbp[:, :])
        o_sb = []
        for co in range(4):
            ot_ = osb.tile([128, CW], BF16, tag="osb")
            if last:
                nc.vector.tensor_mul(out=ot_[:, :], in0=o_ps[co][:, :], in1=rsb[:, :])
            else:
                nc.vector.tensor_copy(out=ot_[:, :], in_=o_ps[co][:, :])
            o_sb.append(ot_)
        for co in range(4):
            prp = ps_o.tile([128, CW], F32, tag="o", name=f"pr{ch}_{co}")
            for c in range(NT):
                nc.tensor.matmul(out=prp[:, :],
                                 lhsT=woT_sb[c][:, co * 128:(co + 1) * 128],
                                 rhs=o_sb[c][:, :],
                                 start=(c == 0), stop=(c == NT - 1))
            ou = outp.tile([128, CW], F32, tag="out")
            if last:
                nc.vector.tensor_add(out=ou[:, :], in0=prp[:, :],
                                     in1=xr_sb[co][:, csl])
            else:
                nc.vector.tensor_mul(out=ou[:, :], in0=prp[:, :], in1=rsb[:, :])
                nc.vector.tensor_add(out=ou[:, :], in0=ou[:, :],
                                     in1=xr_sb[co][:, csl])
            nc.sync.dma_start(out=d["out"][co * 128:(co + 1) * 128, csl], in_=ou[:, :])

    for p in (smsb, outp, osb, pp, ps_o, ps_mm, vtp, qkp, vecs, mp,
              wvo, wqk, rp, xp):
        p.release()


def _sel_consts():
    sel = np.zeros((128, GPT), np.float32)
    for p in range(128):
        sel[p, p // 16] = 1.0
    return sel, np.ascontiguousarray(sel.T)


def kernel(x, gn_w, gn_b, wq, bq, wk, bk, wv, bv, wo, bo):
    del bk  # exactly cancelled by softmax shift invariance
    if "nc" not in _CACHE:
        _CACHE["nc"] = _build_bass()
    nc = _CACHE["nc"]

    bf = ml_dtypes.bfloat16
    x = np.ascontiguousarray(np.asarray(x, np.float32)).reshape(B, C, N)
    wqb = np.ascontiguousarray(np.asarray(wq, np.float32).astype(bf))
    wkb = np.ascontiguousarray(np.asarray(wk, np.float32).astype(bf))
    wvT = np.ascontiguousarray(np.asarray(wv, np.float32).T.astype(bf))
    woT = np.ascontiguousarray(np.asarray(wo, np.float32).T.astype(bf))
    vecs = {n: np.ascontiguousarray(np.asarray(v, np.float32))
            for n, v in (("gnw", gn_w), ("gnb", gn_b), ("bv", bv), ("bo", bo))}
    vecs["bq"] = np.ascontiguousarray(np.asarray(bq, np.float32).astype(bf))
    sel, selT = _sel_consts()

    in_maps = []
    for core in range(8):
        b, qb = core // 4, core % 4
        xb = np.ascontiguousarray(np.roll(x[b], -qb * NQ, axis=1))
        in_maps.append({"x": xb.astype(bf),
                        "xr": np.ascontiguousarray(xb[:, :NQ]),
                        "wq": wqb, "wk": wkb, "wvT": wvT, "woT": woT,
                        "sel": sel, "selT": selT, **vecs})

    _CACHE["last_in_maps"] = in_maps
    res = run_bass_kernel_spmd(nc, in_maps, list(range(8))).results
    out = np.empty((B, C, N), np.float32)
    for core in range(8):
        b, qb = core // 4, core % 4
        out[b][:, qb * NQ:(qb + 1) * NQ] = res[core]["out"]
    return out.reshape(B, C, HH, WW)
